# revision 32
# baseline (speedup 1.0000x reference)
"""BiLSTM-CRF loss kernel for 8 Trainium2 NeuronCores.

Sharding: data-parallel over batch (64 -> 8 cores x 8 rows). Each core runs
both LSTM directions for its batch shard, computes CRF emissions, runs the
CRF forward pass in the exp domain, the gold-path score, and writes the
partial sum of (forward - gold) over its 8 rows. Host sums partials / 64.

Key device-side tricks:
  - All gate nonlinearities collapse to a single tanh per step:
    sigmoid(x) = 0.5*(1+tanh(x/2)); the 0.5 input scales are folded into
    pre-scaled weight copies on the host, and doubled state (C=2c, H=2h)
    absorbs the output scales.
  - LSTM works in a transposed layout: stationary operand = Whh chunks,
    moving operand = H^T, so the pointwise math runs on all 128 partitions.
  - CRF forward runs in the exp domain: A_t = E_t * (P @ A_{t-1}) with
    P = exp(transitions)^T resident as the PE stationary operand, a ones
    column appended to P to produce running sums for periodic renorm.
"""

import sys

sys.path.insert(0, "/opt/trn_rl_repo")

import numpy as np
import ml_dtypes

import concourse.bass as bass
from concourse import bacc
import concourse.tile as tile
from concourse import mybir
from concourse.bass import IndirectOffsetOnAxis
from concourse import bass_isa
from concourse.bass_utils import run_bass_kernel_spmd
from concourse.masks import make_identity

F32 = mybir.dt.float32
BF16 = mybir.dt.bfloat16
I32 = mybir.dt.int32
ALU = mybir.AluOpType
AF = mybir.ActivationFunctionType
AX = mybir.AxisListType

B, L, E, H, C = 64, 256, 256, 256, 20
G = 4 * H
NCORES = 8
BC = B // NCORES            # batch rows per core
CH = 8                      # gate-hidden chunks of 128 (c = gate*2 + half)
NT = (L * BC) // 128        # token tiles per direction = 16
TPT = 128 // BC             # timesteps per token tile = 16
REN = 8                     # CRF renorm interval
NREN = L // REN
NSL = 64                    # renorm log slots (31 fwd, 31 bwd, 1 final)
START, STOP = 18, 19

_CACHE = {}


def _build_module():
    nc = bacc.Bacc(None, target_bir_lowering=False, debug=False)

    # ---- DRAM I/O ----
    d_embed = nc.dram_tensor("embed_bf", [50000, E], BF16, kind="ExternalInput")
    d_idxf = nc.dram_tensor("idx_f", [128, NT], I32, kind="ExternalInput")
    d_idxb = nc.dram_tensor("idx_b", [128, NT], I32, kind="ExternalInput")
    d_wih = nc.dram_tensor("wih", [128, 2, 2, CH, 128], BF16, kind="ExternalInput")
    d_whh = nc.dram_tensor("whh", [128, 2, 2, CH, 128], BF16, kind="ExternalInput")
    d_xbias = nc.dram_tensor("xbias", [128, 2, CH], F32, kind="ExternalInput")
    d_h0 = nc.dram_tensor("h0T", [128, 2, 2, BC], BF16, kind="ExternalInput")
    d_c0 = nc.dram_tensor("c0T", [128, 2, 2, BC], F32, kind="ExternalInput")
    d_wout = nc.dram_tensor("woutT", [128, 2, 2, C], BF16, kind="ExternalInput")
    d_bout = nc.dram_tensor("bout", [C, 1], F32, kind="ExternalInput")
    d_transT = nc.dram_tensor("transT", [C, C], F32, kind="ExternalInput")
    d_tstop = nc.dram_tensor("tstop", [C, 1], F32, kind="ExternalInput")
    d_ohprev = nc.dram_tensor("ohprev", [C, BC, L], F32, kind="ExternalInput")
    d_ohcur = nc.dram_tensor("ohcur", [C, BC, L], F32, kind="ExternalInput")
    d_ohlast = nc.dram_tensor("ohlast", [C, BC], F32, kind="ExternalInput")
    d_a0 = nc.dram_tensor("a0", [C, BC], F32, kind="ExternalInput")
    d_out = nc.dram_tensor("out", [1, 1], F32, kind="ExternalOutput")

    with tile.TileContext(nc) as tc:
        with (
            tc.tile_pool(name="persist", bufs=1) as pp,
            tc.tile_pool(name="work", bufs=3) as wp,
            tc.tile_pool(name="lstm", bufs=3) as lp,
        ):
            # ---- persistent SBUF ----
            wih_sb = pp.tile([128, 2, 2, CH, 128], BF16, tag="wih")
            whh_sb = pp.tile([128, 2, 2, CH, 128], BF16, tag="whh")
            xbias_sb = pp.tile([128, 2, CH], F32, tag="xbias")
            wout_sb = pp.tile([128, 2, 2, C], BF16, tag="wout")
            bout_sb = pp.tile([C, 1], F32, tag="bout")
            transT_sb = pp.tile([C, C], F32, tag="transT")
            tstop_sb = pp.tile([C, 1], F32, tag="tstop")
            ohprev_sb = pp.tile([C, BC, L], F32, tag="ohprev")
            ohcur_sb = pp.tile([C, BC, L], F32, tag="ohcur")
            ohlast_sb = pp.tile([C, BC], F32, tag="ohlast")
            idxf_sb = pp.tile([128, NT], I32, tag="idxf")
            idxb_sb = pp.tile([128, NT], I32, tag="idxb")
            ident128 = pp.tile([128, 128], BF16, tag="id128")
            ident20 = pp.tile([C, C], F32, tag="id20")
            s_tiles = pp.tile([128, NT], F32, tag="stiles")
            s_row = pp.tile([1, L * BC], F32, tag="srow")
            # x^T staging: [E-part, k-half of E? no: k index of E chunk, tile, token]
            xTf = pp.tile([128, 2, NT, 128], BF16, tag="xTf")
            xTb = pp.tile([128, 2, NT, 128], BF16, tag="xTb")
            # xp^T: [ghid-part, t, chunk, b]
            xpT = [pp.tile([128, L, CH, BC], BF16, name=f"xpT{d}", tag=f"xpT{d}") for d in (0, 1)]
            # H history: [hid-part, k-half, t(0..L), b]   slot s+1 = state after step s
            hsT = [pp.tile([128, 2, L + 1, BC], BF16, name=f"hsT{d}", tag=f"hsT{d}") for d in (0, 1)]
            cst = [pp.tile([128, 2, BC], F32, name=f"cst{d}", tag=f"cst{d}") for d in (0, 1)]
            featsT = pp.tile([128, L, BC], F32, tag="featsT")
            eT = pp.tile([C, L, BC], F32, tag="eT")
            pplus = pp.tile([C, C], F32, tag="pplus")
            pplusT = pp.tile([C, C], F32, tag="pplusT")
            wstop = pp.tile([C, 1], F32, tag="wstop")
            sall = pp.tile([1, BC, NSL], F32, tag="sall")
            ones1 = pp.tile([1, C], F32, tag="ones1")
            ones20c = pp.tile([C, 1], F32, tag="ones20c")
            avec = pp.tile([C, 2, BC], F32, tag="avec")
            gold_sb = pp.tile([1, BC], F32, tag="gold")

            # ---- load constants ----
            nc.sync.dma_start(out=wih_sb[:], in_=d_wih[:])
            nc.sync.dma_start(out=whh_sb[:], in_=d_whh[:])
            nc.sync.dma_start(out=xbias_sb[:], in_=d_xbias[:])
            nc.sync.dma_start(out=wout_sb[:], in_=d_wout[:])
            nc.sync.dma_start(out=bout_sb[:], in_=d_bout[:])
            nc.sync.dma_start(out=transT_sb[:], in_=d_transT[:])
            nc.sync.dma_start(out=tstop_sb[:], in_=d_tstop[:])
            nc.sync.dma_start(out=ohprev_sb[:], in_=d_ohprev[:])
            nc.sync.dma_start(out=ohcur_sb[:], in_=d_ohcur[:])
            nc.sync.dma_start(out=ohlast_sb[:], in_=d_ohlast[:])
            nc.sync.dma_start(out=idxf_sb[:], in_=d_idxf[:])
            nc.sync.dma_start(out=idxb_sb[:], in_=d_idxb[:])
            for d in (0, 1):
                nc.sync.dma_start(out=hsT[d][:, :, 0, :], in_=d_h0[:, d, :, :])
                nc.sync.dma_start(out=cst[d][:], in_=d_c0[:, d, :, :])
            make_identity(nc, ident128[:])
            make_identity(nc, ident20[:])
            nc.vector.memset(ones1[:], 1.0)
            nc.vector.memset(ones20c[:], 1.0)
            nc.vector.memset(sall[:], 1.0)  # unwritten renorm slots -> Ln()=0

            # DVE staging copies so downstream DVE ops carry <=1 sync wait
            ohcur_c = pp.tile([C, BC, L], F32, tag="ohcur_c")
            nc.vector.tensor_copy(ohcur_c[:], ohcur_sb[:])
            ohlast_c = pp.tile([C, BC], F32, tag="ohlast_c")
            nc.vector.tensor_copy(ohlast_c[:], ohlast_sb[:])

            # P+ = exp(transT);  wstop = exp(T[STOP,:])
            nc.scalar.activation(pplus[:], transT_sb[:], AF.Exp)
            nc.scalar.activation(wstop[:], tstop_sb[:], AF.Exp)

            # A0 = onehot(START) in slot 0
            nc.vector.memset(avec[:], 0.0)
            nc.sync.dma_start(out=avec[:, 0, :], in_=d_a0[:])

            # ---- P1: gather + transpose + input projection ----
            ps_p1 = tc.tile_pool(name="ps_p1", bufs=2, space="PSUM")
            psA = psB = ps_p1.__enter__()
            # pplusT = exp(trans) in natural layout (stationary for the beta
            # chain's P^T matmul in P5)
            ptp = psA.tile([C, C], F32, tag="ptp")
            nc.tensor.transpose(ptp[:], pplus[:], ident20[:])
            nc.vector.tensor_copy(pplusT[:], ptp[:])
            for d in (0, 1):
                idx_sb = idxf_sb if d == 0 else idxb_sb
                xT = xTf if d == 0 else xTb
                for ti in range(NT):
                    gx = wp.tile([128, E], BF16, tag="gx")
                    nc.gpsimd.indirect_dma_start(
                        out=gx[:],
                        out_offset=None,
                        in_=d_embed[:],
                        in_offset=IndirectOffsetOnAxis(ap=idx_sb[:, ti : ti + 1], axis=0),
                    )
                    for k in (0, 1):
                        pt = psB.tile([128, 128], BF16, tag="ptr")
                        nc.tensor.transpose(
                            pt[:], gx[:, k * 128 : (k + 1) * 128], ident128[:]
                        )
                        nc.vector.tensor_copy(xT[:, k, ti, :], pt[:])

            nco = 0
            for d in (0, 1):
                xT = xTf if d == 0 else xTb
                for c in range(CH):
                    for h in (0, 1):
                        pj = psA.tile([128, 8, 128], F32, tag="pj", bufs=2)
                        for tj in range(8):
                            for k in (0, 1):
                                nc.tensor.matmul(
                                    pj[:, tj, :],
                                    wih_sb[:, d, k, c, :],
                                    xT[:, k, h * 8 + tj, :],
                                    start=(k == 0),
                                    stop=(k == 1),
                                )
                        # xp^T[t, c, b] = pj + bias; copy-out with the bias
                        # folded in, alternating Act (Identity w/ per-partition
                        # bias) and DVE (tensor_scalar) to balance engines
                        nco += 1
                        if nco % 3 != 0:
                            nc.scalar.activation(
                                xpT[d][:, h * 128 : (h + 1) * 128, c, :],
                                pj[:].rearrange("p tj (tl b) -> p (tj tl) b", b=BC),
                                AF.Identity,
                                bias=xbias_sb[:, d, c : c + 1],
                            )
                        else:
                            nc.vector.tensor_scalar(
                                out=xpT[d][:, h * 128 : (h + 1) * 128, c, :],
                                in0=pj[:].rearrange(
                                    "p tj (tl b) -> p (tj tl) b", b=BC
                                ),
                                scalar1=xbias_sb[:, d, c : c + 1],
                                scalar2=None,
                                op0=ALU.add,
                            )
            ps_p1.__exit__(None, None, None)

            # ---- P2: LSTM recurrence (both directions interleaved) ----
            ps_p2 = tc.tile_pool(name="ps_p2", bufs=4, space="PSUM")
            psB = ps_p2.__enter__()
            th2 = [lp.tile([128, CH + 2, BC], F32, name=f"th_init{d}", tag="th", bufs=6) for d in (0, 1)]
            for d in (0, 1):
                nc.vector.tensor_copy(th2[d][:, 8:10, :], cst[d][:])
            for s in range(L):
                # Gate chunk order is (o, i, f, g); the cell state C lives
                # in chunks 8:10 of the th tile (written by the previous
                # step's c-update) so (g|C) is one contiguous STT operand.
                pg2, tcc2, w2, thn = [], [], [], [None, None]
                for d in (0, 1):
                    pg = psB.tile([128, CH, BC], F32, tag="pg")
                    # xp (incl. bias) accumulated via identity matmul; only
                    # depends on xpT so PE can run it ahead of h.
                    nc.tensor.matmul(
                        pg[:].rearrange("p c b -> p (c b)"),
                        ident128[:],
                        xpT[d][:, s, :, :].rearrange("p c b -> p (c b)"),
                        start=True,
                        stop=False,
                        skip_group_check=True,
                    )
                    for k in (0, 1):
                        for c in range(CH):
                            nc.tensor.matmul(
                                pg[:, c, :],
                                whh_sb[:, d, k, c, :],
                                hsT[d][:, k, s, :],
                                start=False,
                                stop=(k == 1),
                                skip_group_check=True,
                            )
                    pg2.append(pg)
                for d in (0, 1):
                    nc.scalar.activation(th2[d][:, 0:CH, :], pg2[d][:], AF.Tanh)
                for d in (0, 1):
                    th = th2[d]
                    w = lp.tile([128, 4, BC], F32, tag="w")
                    # w[0:2] = (1+th_i)*th_g ; w[2:4] = (1+th_f)*C
                    nc.vector.scalar_tensor_tensor(
                        out=w[:], in0=th[:, 2:6, :], scalar=1.0, in1=th[:, 6:10, :],
                        op0=ALU.add, op1=ALU.mult,
                    )
                    w2.append(w)
                    thn[d] = lp.tile([128, CH + 2, BC], F32, name=f"thn{d}", tag="th", bufs=6)
                    nc.vector.scalar_tensor_tensor(
                        out=thn[d][:, 8:10, :], in0=w[:, 2:4, :], scalar=0.5,
                        in1=w[:, 0:2, :], op0=ALU.mult, op1=ALU.add,
                    )
                for d in (0, 1):
                    tcc = lp.tile([128, 2, BC], F32, tag="tcc")
                    nc.scalar.activation(tcc[:], thn[d][:, 8:10, :], AF.Tanh, scale=0.5)
                    tcc2.append(tcc)
                for d in (0, 1):
                    # split by k-half so next step's k=0 Whh matmuls can
                    # start before the full h vector lands
                    for k in (0, 1):
                        nc.vector.scalar_tensor_tensor(
                            out=hsT[d][:, k, s + 1, :], in0=th2[d][:, k, :],
                            scalar=1.0, in1=tcc2[d][:, k, :],
                            op0=ALU.add, op1=ALU.mult,
                        )
                th2 = thn

            ps_p2.__exit__(None, None, None)

            # ---- P3: emissions feats^T = sum_d Wout_d @ H_d + bout ----
            ps_p3 = tc.tile_pool(name="ps_p3", bufs=1, space="PSUM")
            psA = psB = ps_p3.__enter__()
            pf = psA.tile([C, L * BC], F32, tag="big")
            for d in (0, 1):
                for k in (0, 1):
                    for n in range(4):
                        nc.tensor.matmul(
                            pf[:, n * 512 : (n + 1) * 512],
                            wout_sb[:, d, k, :],
                            hsT[d][:, k, 1 + n * 64 : 1 + (n + 1) * 64, :],
                            start=(d == 0 and k == 0),
                            stop=(d == 1 and k == 1),
                        )
            nc.scalar.activation(
                featsT[0:C].rearrange("p t b -> p (t b)"),
                pf[:],
                AF.Identity,
                bias=bout_sb[:, 0:1],
            )

            # ---- P4: CRF prep ----
            # per-(t,b) max over tags via PE transpose + free-dim reduce
            for ti in range(NT):
                ptf = psB.tile([128, C], F32, tag="ptf", bufs=2)
                nc.tensor.transpose(
                    ptf[:], featsT[0:C, ti * TPT : (ti + 1) * TPT, :], ident20[:]
                )
                nc.vector.tensor_reduce(
                    out=s_tiles[:, ti : ti + 1], in_=ptf[:], axis=AX.X, op=ALU.max
                )

            # s_row[0, p*NT + ti] = s_tiles[p, ti]  (one DMA, one sem lane)
            nc.sync.dma_start(out=s_row[0:1, :], in_=s_tiles[:])
            # broadcast s over tag partitions (ones-column matmul); rhs view
            # reorders (tl, b, ti) -> feats order (ti, tl, b)
            sv = s_row[0:1, :].rearrange(
                "a (tl b ti) -> a ti tl b", b=BC, ti=NT
            )
            psb = psA.tile([C, L * BC], F32, tag="big")
            for n in range(4):
                nc.tensor.matmul(
                    psb[:, n * 512 : (n + 1) * 512],
                    ones1[:],
                    sv[:, n * 4 : (n + 1) * 4, :, :],
                    start=True,
                    stop=True,
                )
            eTv = eT[:].rearrange("p t b -> p (t b)")
            nc.vector.scalar_tensor_tensor(
                out=eTv, in0=featsT[0:C].rearrange("p t b -> p (t b)"), scalar=0.0, in1=psb[:], op0=ALU.add, op1=ALU.subtract,
            )
            nc.scalar.activation(eTv, eTv, AF.Exp)

            # gold score: U = T @ ohprev ; transum = sum_t (U * ohcur)
            pu = psA.tile([C, BC * L], F32, tag="big")
            for n in range(4):
                nc.tensor.matmul(
                    pu[:, n * 512 : (n + 1) * 512],
                    transT_sb[:],
                    ohprev_sb[:].rearrange("p b t -> p (b t)")[
                        :, n * 512 : (n + 1) * 512
                    ],
                    start=True,
                    stop=True,
                )
            prod = pp.tile([C, BC, L], F32, tag="prod")
            nc.vector.scalar_tensor_tensor(
                out=prod[:].rearrange("p b t -> p (b t)"), in0=pu[:], scalar=0.0, in1=ohcur_c[:].rearrange("p b t -> p (b t)"), op0=ALU.add, op1=ALU.mult,
            )
            gsum = pp.tile([C, BC], F32, tag="gsum")
            nc.vector.tensor_reduce(out=gsum[:], in_=prod[:], axis=AX.X, op=ALU.add)
            # emissions at gold tags: featsT viewed [C, b, t] * ohcur
            prod2 = pp.tile([C, BC, L], F32, tag="prod2")
            nc.gpsimd.tensor_tensor(
                out=prod2[:], in0=featsT[0:C].rearrange("p t b -> p b t"),
                in1=ohcur_c[:], op=ALU.mult,
            )
            gsum2 = pp.tile([C, BC], F32, tag="gsum2")
            nc.vector.tensor_reduce(out=gsum2[:], in_=prod2[:], axis=AX.X, op=ALU.add)
            nc.vector.scalar_tensor_tensor(
                out=gsum[:], in0=gsum[:], scalar=0.0, in1=gsum2[:], op0=ALU.add, op1=ALU.add,
            )
            # + T[STOP, tag_last]
            stoption = pp.tile([C, BC], F32, tag="stopterm")
            nc.vector.scalar_tensor_tensor(
                out=stoption[:], in0=ohlast_c[:], scalar=0.0, in1=tstop_sb[:].to_broadcast([C, BC]), op0=ALU.add, op1=ALU.mult,
            )
            nc.vector.scalar_tensor_tensor(
                out=gsum[:], in0=gsum[:], scalar=0.0, in1=stoption[:], op0=ALU.add, op1=ALU.add,
            )
            pgold = psB.tile([1, BC], F32, tag="pgold")
            nc.tensor.matmul(pgold[:], ones20c[:], gsum[:], start=True, stop=True)
            nc.vector.tensor_copy(gold_sb[:], pgold[:])

            ps_p3.__exit__(None, None, None)

            # ---- P5: CRF forward scan (exp domain), meet-in-the-middle ----
            # alpha chain (DVE) consumes E_0..E_{MID-1} left-to-right; beta
            # chain (Pool) consumes E_{L-1}..E_{MID} right-to-left with
            # beta_L = wstop, beta_t = P^T (E_{t+1} o beta_{t+1}).  Final
            # Z = beta_mid^T alpha_mid.  Renorms run OFF the chain: the
            # 1/colsum scale computed at round r is folded into the eT slice
            # consumed at round r+REN (linear recurrence, scales commute);
            # only applied scales are logged in sall (slots 0..14 fwd,
            # 16..30 bwd; 15/31 stay 1.0).
            MID = L // 2
            RENF = 4
            ps_p5 = tc.tile_pool(name="ps_p5", bufs=2, space="PSUM")
            psB = ps_p5.__enter__()
            pb2_prev = None
            for r in range(MID):
                ta, tb = r, L - 1 - r
                # alpha round
                pa = psB.tile([C, BC], F32, tag="pa")
                nc.tensor.matmul(
                    pa[:], pplus[:], avec[:, r % 2, :], start=True, stop=True
                )
                nc.vector.scalar_tensor_tensor(
                    out=avec[:, (r + 1) % 2, :], in0=pa[:], scalar=0.0,
                    in1=eT[:, ta, :], op0=ALU.add, op1=ALU.mult,
                )
                # beta round: w = E_tb o beta (Pool, PSUM in), then P^T @ w
                wb = wp.tile([C, BC], F32, tag="wb")
                bin_ = wstop[:].to_broadcast([C, BC]) if r == 0 else pb2_prev[:]
                nc.vector.scalar_tensor_tensor(
                    out=wb[:], in0=bin_, scalar=0.0,
                    in1=eT[:, tb, :], op0=ALU.add, op1=ALU.mult,
                )
                pb2 = psB.tile([C, BC], F32, tag="pb2")
                nc.tensor.matmul(pb2[:], pplusT[:], wb[:], start=True, stop=True)
                pb2_prev = pb2
                if r % RENF == RENF - 1 and r + RENF < MID:
                    rn = r // RENF
                    # fwd renorm (off-chain, DVE)
                    pss = psB.tile([1, BC], F32, tag="pss", bufs=2)
                    nc.tensor.matmul(
                        pss[:], ones20c[:], avec[:, (r + 1) % 2, :],
                        start=True, stop=True,
                    )
                    nc.scalar.activation(sall[0:1, :, rn], pss[:], AF.Copy)
                    srec = wp.tile([1, BC], F32, tag="srec")
                    nc.vector.reciprocal(srec[:], pss[:])
                    pb = psB.tile([C, BC], F32, tag="pb", bufs=1)
                    nc.tensor.matmul(pb[:], ones1[:], srec[:], start=True, stop=True)
                    nc.vector.scalar_tensor_tensor(
                        out=eT[:, ta + RENF, :], in0=eT[:, ta + RENF, :],
                        scalar=0.0, in1=pb[:], op0=ALU.add, op1=ALU.mult,
                    )
                    # bwd renorm (off-chain, Pool), logged from the SBUF wb
                    pss2 = psB.tile([1, BC], F32, tag="pss", bufs=2)
                    nc.tensor.matmul(pss2[:], ones20c[:], wb[:], start=True, stop=True)
                    nc.scalar.activation(sall[0:1, :, 32 + rn], pss2[:], AF.Copy)
                    srec2 = wp.tile([1, BC], F32, tag="srec2")
                    nc.vector.reciprocal(srec2[:], pss2[:])
                    pbb = psB.tile([C, BC], F32, tag="pb", bufs=1)
                    nc.tensor.matmul(pbb[:], ones1[:], srec2[:], start=True, stop=True)
                    nc.vector.scalar_tensor_tensor(
                        out=eT[:, tb - RENF, :], in0=eT[:, tb - RENF, :],
                        scalar=0.0, in1=pbb[:], op0=ALU.add, op1=ALU.mult,
                    )

            # ---- P6: finalization  Z = beta_mid^T alpha_mid ----
            # normalize by colsum(alpha_mid) (logged at slot 31) so the Ln
            # input stays inside the Scalar-engine range
            pssA = psB.tile([1, BC], F32, tag="pss", bufs=2)
            nc.tensor.matmul(
                pssA[:], ones20c[:], avec[:, MID % 2, :], start=True, stop=True
            )
            nc.scalar.activation(sall[0:1, :, 31], pssA[:], AF.Copy)
            srecA = wp.tile([1, BC], F32, tag="srecA")
            nc.vector.reciprocal(srecA[:], pssA[:])
            pbA = psB.tile([C, BC], F32, tag="pb", bufs=1)
            nc.tensor.matmul(pbA[:], ones1[:], srecA[:], start=True, stop=True)
            prodZ = wp.tile([C, BC], F32, tag="prodZ")
            nc.vector.scalar_tensor_tensor(
                out=prodZ[:], in0=pb2_prev[:], scalar=0.0,
                in1=avec[:, MID % 2, :], op0=ALU.add, op1=ALU.mult,
            )
            nc.vector.scalar_tensor_tensor(
                out=prodZ[:], in0=prodZ[:], scalar=0.0,
                in1=pbA[:], op0=ALU.add, op1=ALU.mult,
            )
            paf = psB.tile([1, BC], F32, tag="paf", bufs=1)
            nc.tensor.matmul(paf[:], ones20c[:], prodZ[:], start=True, stop=True)
            flog = wp.tile([1, BC], F32, tag="flog")
            nc.scalar.activation(flog[:], paf[:], AF.Ln)
            slog = wp.tile([1, BC, NSL], F32, tag="slog")
            nc.scalar.activation(slog[:], sall[:], AF.Ln)
            slogsum = wp.tile([1, BC], F32, tag="slogsum")
            nc.vector.tensor_reduce(out=slogsum[:], in_=slog[:], axis=AX.X, op=ALU.add)
            # sum of per-step shifts s_t
            ssum = wp.tile([1, BC], F32, tag="ssum")
            nc.vector.tensor_reduce(
                out=ssum[:],
                in_=s_row[0:1, :].rearrange("a (tl b ti) -> a b tl ti", b=BC, ti=NT),
                axis=AX.XY,
                op=ALU.add,
            )
            # F - gold, sum over batch
            fsum = wp.tile([1, BC], F32, tag="fsum")
            nc.vector.scalar_tensor_tensor(
                out=fsum[:], in0=flog[:], scalar=0.0, in1=slogsum[:], op0=ALU.add, op1=ALU.add,
            )
            nc.vector.scalar_tensor_tensor(
                out=fsum[:], in0=fsum[:], scalar=0.0, in1=ssum[:], op0=ALU.add, op1=ALU.add,
            )
            nc.vector.scalar_tensor_tensor(
                out=fsum[:], in0=fsum[:], scalar=0.0, in1=gold_sb[:], op0=ALU.add, op1=ALU.subtract,
            )
            lp_t = wp.tile([1, 1], F32, tag="lp")
            nc.vector.tensor_reduce(out=lp_t[:], in_=fsum[:], axis=AX.X, op=ALU.add)
            nc.sync.dma_start(out=d_out[:], in_=lp_t[:])
            ps_p5.__exit__(None, None, None)

    nc.finalize()
    return nc


def _prep_inmaps(inputs):
    bf = ml_dtypes.bfloat16
    sent = np.asarray(inputs["sentences"])
    tags = np.asarray(inputs["tags"])
    embed = np.asarray(inputs["embed"], dtype=np.float32)
    trans = np.asarray(inputs["transitions"], dtype=np.float32)
    h0 = np.asarray(inputs["h0"], dtype=np.float32)
    c0 = np.asarray(inputs["c0"], dtype=np.float32)
    W_out = np.asarray(inputs["W_out"], dtype=np.float32)
    b_out = np.asarray(inputs["b_out"], dtype=np.float32)

    rs = np.full((G, 1), 0.5, np.float32)
    rs[2 * H : 3 * H] = 1.0  # g-gate rows unscaled

    embed_bf = np.ascontiguousarray(embed.astype(bf))

    GP = [3, 0, 1, 2]  # gate order (o, i, f, g)

    def chunk_weights(W):  # W [G, K_in] -> [128, 2, CH, 128] = [p, k, c, m]
        Kin = W.shape[1]
        Wr = W.reshape(4, 2, 128, Kin // 128, 128)[GP]  # [gate, hh, m, k, p]
        return np.ascontiguousarray(Wr.transpose(4, 3, 0, 1, 2).reshape(128, Kin // 128, CH, 128))

    wih = np.zeros((128, 2, 2, CH, 128), np.float32)
    whh = np.zeros((128, 2, 2, CH, 128), np.float32)
    xbias = np.zeros((128, 2, CH), np.float32)  # broadcast to BC below
    for d, (Wih, Whh, b) in enumerate(
        [
            (inputs["Wih_f"], inputs["Whh_f"], inputs["b_f"]),
            (inputs["Wih_b"], inputs["Whh_b"], inputs["b_b"]),
        ]
    ):
        Wih = np.asarray(Wih, np.float32) * rs
        Whh = np.asarray(Whh, np.float32) * rs * 0.5
        bt = np.asarray(b, np.float32) * rs[:, 0]
        wih[:, d] = chunk_weights(Wih)
        whh[:, d] = chunk_weights(Whh)
        xbias[:, d] = bt.reshape(4, 2, 128)[GP].transpose(2, 0, 1).reshape(128, CH)
    wih = np.ascontiguousarray(wih.astype(bf))
    whh = np.ascontiguousarray(whh.astype(bf))

    # wout^T [p, d, k, m] = 0.5 * W_out[m, d*256 + k*128 + p]
    wout = np.ascontiguousarray(
        (0.5 * W_out).reshape(C, 2, 2, 128).transpose(3, 1, 2, 0).astype(bf)
    )
    bout = np.ascontiguousarray(b_out[:, None])
    transT = np.ascontiguousarray(trans.T)
    tstop = np.ascontiguousarray(trans[STOP, :][:, None])

    in_maps = []
    for q in range(NCORES):
        bs = slice(q * BC, (q + 1) * BC)
        sq = sent[bs]  # [BC, L]
        tq = tags[bs]
        idx_f = np.ascontiguousarray(
            sq.T.reshape(NT, TPT, BC).transpose(1, 2, 0).reshape(128, NT).astype(np.int32)
        )
        sqr = sq[:, ::-1]
        idx_b = np.ascontiguousarray(
            sqr.T.reshape(NT, TPT, BC).transpose(1, 2, 0).reshape(128, NT).astype(np.int32)
        )
        h0q = np.ascontiguousarray(
            (2.0 * h0[:, bs, :]).reshape(2, BC, 2, 128).transpose(3, 0, 2, 1).astype(bf)
        )
        c0q = np.ascontiguousarray(
            (2.0 * c0[:, bs, :]).reshape(2, BC, 2, 128).transpose(3, 0, 2, 1).astype(np.float32)
        )
        te_prev = np.concatenate(
            [np.full((BC, 1), START, tags.dtype), tq[:, :-1]], axis=1
        )  # prev tag at each t
        ar = np.arange(C)
        ohprev = (ar[:, None, None] == te_prev[None, :, :]).astype(np.float32)
        ohcur = (ar[:, None, None] == tq[None, :, :]).astype(np.float32)
        ohlast = (ar[:, None] == tq[None, :, L - 1]).astype(np.float32)
        a0 = (ar[:, None] == START).astype(np.float32) * np.ones((1, BC), np.float32)
        in_maps.append(
            {
                "embed_bf": embed_bf,
                "idx_f": idx_f,
                "idx_b": idx_b,
                "wih": wih,
                "whh": whh,
                "xbias": xbias,
                "h0T": h0q,
                "c0T": c0q,
                "woutT": wout,
                "bout": bout,
                "transT": transT,
                "tstop": tstop,
                "ohprev": np.ascontiguousarray(ohprev),
                "ohcur": np.ascontiguousarray(ohcur),
                "ohlast": np.ascontiguousarray(ohlast),
                "a0": np.ascontiguousarray(a0),
            }
        )
    return in_maps


def get_module():
    if "nc" not in _CACHE:
        _CACHE["nc"] = _build_module()
    return _CACHE["nc"]


def kernel(**inputs):
    nc = get_module()
    in_maps = _prep_inmaps(inputs)
    res = run_bass_kernel_spmd(nc, in_maps, core_ids=list(range(NCORES)))
    total = sum(float(r["out"][0, 0]) for r in res.results)
    return np.float32(total / B)



# revision 38
# speedup vs baseline: 1.0502x; 1.0502x over previous
"""BiLSTM-CRF loss kernel for 8 Trainium2 NeuronCores.

Sharding: data-parallel over batch (64 -> 8 cores x 8 rows). Each core runs
both LSTM directions for its batch shard, computes CRF emissions, runs the
CRF forward pass in the exp domain, the gold-path score, and writes the
partial sum of (forward - gold) over its 8 rows. Host sums partials / 64.

Key device-side tricks:
  - All gate nonlinearities collapse to a single tanh per step:
    sigmoid(x) = 0.5*(1+tanh(x/2)); the 0.5 input scales are folded into
    pre-scaled weight copies on the host, and doubled state (C=2c, H=2h)
    absorbs the output scales.
  - LSTM works in a transposed layout: stationary operand = Whh chunks,
    moving operand = H^T, so the pointwise math runs on all 128 partitions.
  - CRF forward runs in the exp domain: A_t = E_t * (P @ A_{t-1}) with
    P = exp(transitions)^T resident as the PE stationary operand, a ones
    column appended to P to produce running sums for periodic renorm.
"""

import sys

sys.path.insert(0, "/opt/trn_rl_repo")

import numpy as np
import ml_dtypes

import concourse.bass as bass
from concourse import bacc
import concourse.tile as tile
from concourse import mybir
from concourse.bass import IndirectOffsetOnAxis
from concourse import bass_isa
from concourse.bass_utils import run_bass_kernel_spmd
from concourse.masks import make_identity

F32 = mybir.dt.float32
BF16 = mybir.dt.bfloat16
I32 = mybir.dt.int32
ALU = mybir.AluOpType
AF = mybir.ActivationFunctionType
AX = mybir.AxisListType

B, L, E, H, C = 64, 256, 256, 256, 20
G = 4 * H
NCORES = 8
BC = B // NCORES            # batch rows per core
CH = 8                      # gate-hidden chunks of 128 (c = gate*2 + half)
NT = (L * BC) // 128        # token tiles per direction = 16
TPT = 128 // BC             # timesteps per token tile = 16
REN = 8                     # CRF renorm interval
NREN = L // REN
NSL = 64                    # renorm log slots (31 fwd, 31 bwd, 1 final)
START, STOP = 18, 19

_CACHE = {}


def _build_module():
    nc = bacc.Bacc(None, target_bir_lowering=False, debug=False)

    # ---- DRAM I/O ----
    d_embed = nc.dram_tensor("embed_bf", [50000, E], BF16, kind="ExternalInput")
    d_idxf = nc.dram_tensor("idx_f", [128, NT], I32, kind="ExternalInput")
    d_idxb = nc.dram_tensor("idx_b", [128, NT], I32, kind="ExternalInput")
    d_wih = nc.dram_tensor("wih", [128, 2, 2, CH, 128], BF16, kind="ExternalInput")
    d_whh = nc.dram_tensor("whh", [128, 2, 2, CH, 128], BF16, kind="ExternalInput")
    d_xbias = nc.dram_tensor("xbias", [128, 2, CH], F32, kind="ExternalInput")
    d_h0 = nc.dram_tensor("h0T", [128, 2, 2, BC], BF16, kind="ExternalInput")
    d_c0 = nc.dram_tensor("c0T", [128, 2, 2, BC], F32, kind="ExternalInput")
    d_wout = nc.dram_tensor("woutT", [128, 2, 2, C], BF16, kind="ExternalInput")
    d_bout = nc.dram_tensor("bout", [C, 1], F32, kind="ExternalInput")
    d_transT = nc.dram_tensor("transT", [C, C], F32, kind="ExternalInput")
    d_tstop = nc.dram_tensor("tstop", [C, 1], F32, kind="ExternalInput")
    d_ohprev = nc.dram_tensor("ohprev", [C, BC, L], F32, kind="ExternalInput")
    d_ohcur = nc.dram_tensor("ohcur", [C, BC, L], F32, kind="ExternalInput")
    d_ohlast = nc.dram_tensor("ohlast", [C, BC], F32, kind="ExternalInput")
    d_a0 = nc.dram_tensor("a0", [C, BC], F32, kind="ExternalInput")
    d_out = nc.dram_tensor("out", [1, 1], F32, kind="ExternalOutput")

    with tile.TileContext(nc) as tc:
        with (
            tc.tile_pool(name="persist", bufs=1) as pp,
            tc.tile_pool(name="work", bufs=3) as wp,
            tc.tile_pool(name="lstm", bufs=3) as lp,
        ):
            # ---- persistent SBUF ----
            wih_sb = pp.tile([128, 2, 2, CH, 128], BF16, tag="wih")
            whh_sb = pp.tile([128, 2, 2, CH, 128], BF16, tag="whh")
            xbias_sb = pp.tile([128, 2, CH], F32, tag="xbias")
            wout_sb = pp.tile([128, 2, 2, C], BF16, tag="wout")
            bout_sb = pp.tile([C, 1], F32, tag="bout")
            transT_sb = pp.tile([C, C], F32, tag="transT")
            tstop_sb = pp.tile([C, 1], F32, tag="tstop")
            ohprev_sb = pp.tile([C, BC, L], F32, tag="ohprev")
            ohcur_sb = pp.tile([C, BC, L], F32, tag="ohcur")
            ohlast_sb = pp.tile([C, BC], F32, tag="ohlast")
            idxf_sb = pp.tile([128, NT], I32, tag="idxf")
            idxb_sb = pp.tile([128, NT], I32, tag="idxb")
            ident128 = pp.tile([128, 128], BF16, tag="id128")
            ident20 = pp.tile([C, C], F32, tag="id20")
            s_tiles = pp.tile([128, NT], F32, tag="stiles")
            s_row = pp.tile([1, L * BC], F32, tag="srow")
            # x^T staging: [E-part, k-half of E? no: k index of E chunk, tile, token]
            xTf = pp.tile([128, 2, NT, 128], BF16, tag="xTf")
            xTb = pp.tile([128, 2, NT, 128], BF16, tag="xTb")
            # xp^T: [ghid-part, t, chunk, b]
            xpT = [pp.tile([128, L, CH, BC], BF16, name=f"xpT{d}", tag=f"xpT{d}") for d in (0, 1)]
            # H history: [hid-part, k-half, t(0..L), b]   slot s+1 = state after step s
            hsT = [pp.tile([128, 2, L + 1, BC], BF16, name=f"hsT{d}", tag=f"hsT{d}") for d in (0, 1)]
            cst = [pp.tile([128, 2, BC], F32, name=f"cst{d}", tag=f"cst{d}") for d in (0, 1)]
            featsT = pp.tile([128, L, BC], F32, tag="featsT")
            eT = pp.tile([C, L, BC], F32, tag="eT")
            pplus = pp.tile([C, C], F32, tag="pplus")
            pplusT = pp.tile([C, C], F32, tag="pplusT")
            wstop = pp.tile([C, 1], F32, tag="wstop")
            sall = pp.tile([1, BC, NSL], F32, tag="sall")
            ones1 = pp.tile([1, C], F32, tag="ones1")
            ones20c = pp.tile([C, 1], F32, tag="ones20c")
            avec = pp.tile([C, 2, BC], F32, tag="avec")
            gold_sb = pp.tile([1, BC], F32, tag="gold")

            # ---- load constants ----
            # order: gather indices first (P1-critical), then LSTM weights,
            # then everything needed only at P3+ (emissions/CRF/gold)
            nc.sync.dma_start(out=idxf_sb[:], in_=d_idxf[:])
            nc.sync.dma_start(out=idxb_sb[:], in_=d_idxb[:])
            nc.sync.dma_start(out=wih_sb[:], in_=d_wih[:])
            nc.sync.dma_start(out=whh_sb[:], in_=d_whh[:])
            nc.sync.dma_start(out=xbias_sb[:], in_=d_xbias[:])
            for d in (0, 1):
                nc.sync.dma_start(out=hsT[d][:, :, 0, :], in_=d_h0[:, d, :, :])
                nc.sync.dma_start(out=cst[d][:], in_=d_c0[:, d, :, :])
            nc.sync.dma_start(out=transT_sb[:], in_=d_transT[:])
            nc.sync.dma_start(out=wout_sb[:], in_=d_wout[:])
            nc.sync.dma_start(out=bout_sb[:], in_=d_bout[:])
            nc.sync.dma_start(out=tstop_sb[:], in_=d_tstop[:])
            nc.sync.dma_start(out=ohprev_sb[:], in_=d_ohprev[:])
            nc.sync.dma_start(out=ohcur_sb[:], in_=d_ohcur[:])
            nc.sync.dma_start(out=ohlast_sb[:], in_=d_ohlast[:])
            make_identity(nc, ident128[:])
            make_identity(nc, ident20[:])
            nc.vector.memset(ones1[:], 1.0)
            nc.vector.memset(ones20c[:], 1.0)
            nc.vector.memset(sall[:], 1.0)  # unwritten renorm slots -> Ln()=0

            # DVE staging copies so downstream DVE ops carry <=1 sync wait
            ohcur_c = pp.tile([C, BC, L], F32, tag="ohcur_c")
            nc.vector.tensor_copy(ohcur_c[:], ohcur_sb[:])
            ohlast_c = pp.tile([C, BC], F32, tag="ohlast_c")
            nc.vector.tensor_copy(ohlast_c[:], ohlast_sb[:])

            # P+ = exp(transT);  wstop = exp(T[STOP,:])
            nc.scalar.activation(pplus[:], transT_sb[:], AF.Exp)
            nc.scalar.activation(wstop[:], tstop_sb[:], AF.Exp)

            # A0 = onehot(START) in slot 0
            nc.vector.memset(avec[:], 0.0)
            nc.sync.dma_start(out=avec[:, 0, :], in_=d_a0[:])

            # ---- P1: gather + transpose + input projection ----
            ps_p1 = tc.tile_pool(name="ps_p1", bufs=2, space="PSUM")
            psA = psB = ps_p1.__enter__()
            # pplusT = exp(trans) in natural layout (stationary for the beta
            # chain's P^T matmul in P5)
            ptp = psA.tile([C, C], F32, tag="ptp")
            nc.tensor.transpose(ptp[:], pplus[:], ident20[:])
            nc.vector.tensor_copy(pplusT[:], ptp[:])
            for d in (0, 1):
                idx_sb = idxf_sb if d == 0 else idxb_sb
                xT = xTf if d == 0 else xTb
                for ti in range(NT):
                    gx = wp.tile([128, E], BF16, tag="gx")
                    nc.gpsimd.indirect_dma_start(
                        out=gx[:],
                        out_offset=None,
                        in_=d_embed[:],
                        in_offset=IndirectOffsetOnAxis(ap=idx_sb[:, ti : ti + 1], axis=0),
                    )
                    for k in (0, 1):
                        pt = psB.tile([128, 128], BF16, tag="ptr")
                        nc.tensor.transpose(
                            pt[:], gx[:, k * 128 : (k + 1) * 128], ident128[:]
                        )
                        nc.vector.tensor_copy(xT[:, k, ti, :], pt[:])

            nco = 0
            for d in (0, 1):
                xT = xTf if d == 0 else xTb
                for c in range(CH):
                    for h in (0, 1):
                        pj = psA.tile([128, 8, 128], F32, tag="pj", bufs=2)
                        for tj in range(8):
                            for k in (0, 1):
                                nc.tensor.matmul(
                                    pj[:, tj, :],
                                    wih_sb[:, d, k, c, :],
                                    xT[:, k, h * 8 + tj, :],
                                    start=(k == 0),
                                    stop=(k == 1),
                                )
                        # xp^T[t, c, b] = pj + bias; copy-out with the bias
                        # folded in, alternating Act (Identity w/ per-partition
                        # bias) and DVE (tensor_scalar) to balance engines
                        nco += 1
                        if nco % 3 != 0:
                            nc.scalar.activation(
                                xpT[d][:, h * 128 : (h + 1) * 128, c, :],
                                pj[:].rearrange("p tj (tl b) -> p (tj tl) b", b=BC),
                                AF.Identity,
                                bias=xbias_sb[:, d, c : c + 1],
                            )
                        else:
                            nc.vector.tensor_scalar(
                                out=xpT[d][:, h * 128 : (h + 1) * 128, c, :],
                                in0=pj[:].rearrange(
                                    "p tj (tl b) -> p (tj tl) b", b=BC
                                ),
                                scalar1=xbias_sb[:, d, c : c + 1],
                                scalar2=None,
                                op0=ALU.add,
                            )
            ps_p1.__exit__(None, None, None)

            # ---- P2: LSTM recurrence (both directions interleaved) ----
            ps_p2 = tc.tile_pool(name="ps_p2", bufs=4, space="PSUM")
            psB = ps_p2.__enter__()
            th2 = [lp.tile([128, CH + 2, BC], F32, name=f"th_init{d}", tag="th", bufs=6) for d in (0, 1)]
            pf = psB.tile([C, L * BC], F32, tag="pf", bufs=1)
            for d in (0, 1):
                nc.vector.tensor_copy(th2[d][:, 8:10, :], cst[d][:])
            for s in range(L):
                # Gate chunk order is (o, i, f, g); the cell state C lives
                # in chunks 8:10 of the th tile (written by the previous
                # step's c-update) so (g|C) is one contiguous STT operand.
                pg2, tcc2, w2, thn = [], [], [], [None, None]
                for d in (0, 1):
                    pg = psB.tile([128, CH, BC], F32, tag="pg", bufs=3)
                    # xp (incl. bias) accumulated via identity matmul; only
                    # depends on xpT so PE can run it ahead of h.
                    nc.tensor.matmul(
                        pg[:].rearrange("p c b -> p (c b)"),
                        ident128[:],
                        xpT[d][:, s, :, :].rearrange("p c b -> p (c b)"),
                        start=True,
                        stop=False,
                        skip_group_check=True,
                    )
                    for k in (0, 1):
                        for c in range(CH):
                            nc.tensor.matmul(
                                pg[:, c, :],
                                whh_sb[:, d, k, c, :],
                                hsT[d][:, k, s, :],
                                start=False,
                                stop=(k == 1),
                                skip_group_check=True,
                            )
                    pg2.append(pg)
                for d in (0, 1):
                    nc.scalar.activation(th2[d][:, 0:CH, :], pg2[d][:], AF.Tanh)
                for d in (0, 1):
                    th = th2[d]
                    w = lp.tile([128, 4, BC], F32, tag="w")
                    # w[0:2] = (1+th_i)*th_g ; w[2:4] = (1+th_f)*C
                    nc.vector.scalar_tensor_tensor(
                        out=w[:], in0=th[:, 2:6, :], scalar=1.0, in1=th[:, 6:10, :],
                        op0=ALU.add, op1=ALU.mult,
                    )
                    w2.append(w)
                    thn[d] = lp.tile([128, CH + 2, BC], F32, name=f"thn{d}", tag="th", bufs=6)
                    nc.vector.scalar_tensor_tensor(
                        out=thn[d][:, 8:10, :], in0=w[:, 2:4, :], scalar=0.5,
                        in1=w[:, 0:2, :], op0=ALU.mult, op1=ALU.add,
                    )
                for d in (0, 1):
                    tcc = lp.tile([128, 2, BC], F32, tag="tcc")
                    nc.scalar.activation(tcc[:], thn[d][:, 8:10, :], AF.Tanh, scale=0.5)
                    tcc2.append(tcc)
                for d in (0, 1):
                    # split by k-half so next step's k=0 Whh matmuls can
                    # start before the full h vector lands
                    for k in (0, 1):
                        nc.vector.scalar_tensor_tensor(
                            out=hsT[d][:, k, s + 1, :], in0=th2[d][:, k, :],
                            scalar=1.0, in1=tcc2[d][:, k, :],
                            op0=ALU.add, op1=ALU.mult,
                        )
                th2 = thn
                # stream emissions blocks: block n needs hsT slots <= 64(n+1),
                # all written once step 64(n+1)-1 is issued
                if s % 64 == 0 and 1 <= s // 64 <= 3:
                    n = s // 64 - 1
                    for d in (0, 1):
                        for k in (0, 1):
                            nc.tensor.matmul(
                                pf[:, n * 512 : (n + 1) * 512],
                                wout_sb[:, d, k, :],
                                hsT[d][:, k, 1 + n * 64 : 1 + (n + 1) * 64, :],
                                start=(d == 0 and k == 0),
                                stop=(d == 1 and k == 1),
                            )
                if s % 64 == 2 and 1 <= s // 64 <= 3:
                    n = s // 64 - 1
                    nc.scalar.activation(
                        featsT[0:C, n * 64 : (n + 1) * 64, :].rearrange(
                            "p t b -> p (t b)"
                        ),
                        pf[:, n * 512 : (n + 1) * 512],
                        AF.Identity,
                        bias=bout_sb[:, 0:1],
                    )
                if s % 64 == 4 and 1 <= s // 64 <= 3:
                    n = s // 64 - 1
                    for ti in range(n * 4, n * 4 + 4):
                        ptf = psB.tile([128, C], F32, tag="ptf", bufs=1)
                        nc.tensor.transpose(
                            ptf[:], featsT[0:C, ti * TPT : (ti + 1) * TPT, :],
                            ident20[:],
                        )
                        nc.vector.tensor_reduce(
                            out=s_tiles[:, ti : ti + 1], in_=ptf[:],
                            axis=AX.X, op=ALU.max,
                        )

            # emissions block 3 (needs the final hsT slots)
            for d in (0, 1):
                for k in (0, 1):
                    nc.tensor.matmul(
                        pf[:, 3 * 512 : 4 * 512],
                        wout_sb[:, d, k, :],
                        hsT[d][:, k, 1 + 3 * 64 : 1 + 4 * 64, :],
                        start=(d == 0 and k == 0),
                        stop=(d == 1 and k == 1),
                    )
            nc.scalar.activation(
                featsT[0:C, 192:256, :].rearrange("p t b -> p (t b)"),
                pf[:, 3 * 512 : 4 * 512],
                AF.Identity,
                bias=bout_sb[:, 0:1],
            )
            ps_p2.__exit__(None, None, None)

            # ---- P3: emissions feats^T = sum_d Wout_d @ H_d + bout ----
            ps_p3 = tc.tile_pool(name="ps_p3", bufs=1, space="PSUM")
            psA = psB = ps_p3.__enter__()

            # ---- P4: CRF prep ----
            # per-(t,b) max over tags via PE transpose + free-dim reduce
            # (blocks 0-2 streamed inside P2; only block 3 here)
            for ti in range(12, NT):
                ptf = psB.tile([128, C], F32, tag="ptf", bufs=3)
                nc.tensor.transpose(
                    ptf[:], featsT[0:C, ti * TPT : (ti + 1) * TPT, :], ident20[:]
                )
                nc.vector.tensor_reduce(
                    out=s_tiles[:, ti : ti + 1], in_=ptf[:], axis=AX.X, op=ALU.max
                )

            # s_row[0, p*NT + ti] = s_tiles[p, ti]  (one DMA, one sem lane)
            nc.sync.dma_start(out=s_row[0:1, :], in_=s_tiles[:])
            # broadcast s over tag partitions (ones-column matmul); rhs view
            # reorders (tl, b, ti) -> feats order (ti, tl, b)
            sv = s_row[0:1, :].rearrange(
                "a (tl b ti) -> a ti tl b", b=BC, ti=NT
            )
            psb = psA.tile([C, L * BC], F32, tag="big")
            for n in range(4):
                nc.tensor.matmul(
                    psb[:, n * 512 : (n + 1) * 512],
                    ones1[:],
                    sv[:, n * 4 : (n + 1) * 4, :, :],
                    start=True,
                    stop=True,
                )
            eTv = eT[:].rearrange("p t b -> p (t b)")
            nc.vector.scalar_tensor_tensor(
                out=eTv, in0=featsT[0:C].rearrange("p t b -> p (t b)"), scalar=0.0, in1=psb[:], op0=ALU.add, op1=ALU.subtract,
            )
            nc.scalar.activation(eTv, eTv, AF.Exp)

            # gold score: U = T @ ohprev ; transum = sum_t (U * ohcur)
            pu = psA.tile([C, BC * L], F32, tag="big")
            for n in range(4):
                nc.tensor.matmul(
                    pu[:, n * 512 : (n + 1) * 512],
                    transT_sb[:],
                    ohprev_sb[:].rearrange("p b t -> p (b t)")[
                        :, n * 512 : (n + 1) * 512
                    ],
                    start=True,
                    stop=True,
                )
            prod = pp.tile([C, BC, L], F32, tag="prod")
            nc.vector.scalar_tensor_tensor(
                out=prod[:].rearrange("p b t -> p (b t)"), in0=pu[:], scalar=0.0, in1=ohcur_c[:].rearrange("p b t -> p (b t)"), op0=ALU.add, op1=ALU.mult,
            )
            gsum = pp.tile([C, BC], F32, tag="gsum")
            nc.vector.tensor_reduce(out=gsum[:], in_=prod[:], axis=AX.X, op=ALU.add)
            # emissions at gold tags: featsT viewed [C, b, t] * ohcur
            prod2 = pp.tile([C, BC, L], F32, tag="prod2")
            nc.gpsimd.tensor_tensor(
                out=prod2[:], in0=featsT[0:C].rearrange("p t b -> p b t"),
                in1=ohcur_c[:], op=ALU.mult,
            )
            gsum2 = pp.tile([C, BC], F32, tag="gsum2")
            nc.vector.tensor_reduce(out=gsum2[:], in_=prod2[:], axis=AX.X, op=ALU.add)
            nc.vector.scalar_tensor_tensor(
                out=gsum[:], in0=gsum[:], scalar=0.0, in1=gsum2[:], op0=ALU.add, op1=ALU.add,
            )
            # + T[STOP, tag_last]
            stoption = pp.tile([C, BC], F32, tag="stopterm")
            nc.vector.scalar_tensor_tensor(
                out=stoption[:], in0=ohlast_c[:], scalar=0.0, in1=tstop_sb[:].to_broadcast([C, BC]), op0=ALU.add, op1=ALU.mult,
            )
            nc.vector.scalar_tensor_tensor(
                out=gsum[:], in0=gsum[:], scalar=0.0, in1=stoption[:], op0=ALU.add, op1=ALU.add,
            )
            pgold = psB.tile([1, BC], F32, tag="pgold")
            nc.tensor.matmul(pgold[:], ones20c[:], gsum[:], start=True, stop=True)
            nc.vector.tensor_copy(gold_sb[:], pgold[:])

            ps_p3.__exit__(None, None, None)

            # ---- P5: CRF forward scan (exp domain), meet-in-the-middle ----
            # alpha chain (DVE) consumes E_0..E_{MID-1} left-to-right; beta
            # chain (Pool) consumes E_{L-1}..E_{MID} right-to-left with
            # beta_L = wstop, beta_t = P^T (E_{t+1} o beta_{t+1}).  Final
            # Z = beta_mid^T alpha_mid.  Renorms run OFF the chain: the
            # 1/colsum scale computed at round r is folded into the eT slice
            # consumed at round r+REN (linear recurrence, scales commute);
            # only applied scales are logged in sall (slots 0..14 fwd,
            # 16..30 bwd; 15/31 stay 1.0).
            MID = L // 2
            RENF = 5
            ps_p5 = tc.tile_pool(name="ps_p5", bufs=2, space="PSUM")
            psB = ps_p5.__enter__()
            pb2_prev = None
            for r in range(MID):
                ta, tb = r, L - 1 - r
                # alpha round
                pa = psB.tile([C, BC], F32, tag="pa")
                nc.tensor.matmul(
                    pa[:], pplus[:], avec[:, r % 2, :], start=True, stop=True
                )
                nc.vector.scalar_tensor_tensor(
                    out=avec[:, (r + 1) % 2, :], in0=pa[:], scalar=0.0,
                    in1=eT[:, ta, :], op0=ALU.add, op1=ALU.mult,
                )
                # beta round: w = E_tb o beta (Pool, PSUM in), then P^T @ w
                wb = wp.tile([C, BC], F32, tag="wb")
                bin_ = wstop[:].to_broadcast([C, BC]) if r == 0 else pb2_prev[:]
                nc.vector.scalar_tensor_tensor(
                    out=wb[:], in0=bin_, scalar=0.0,
                    in1=eT[:, tb, :], op0=ALU.add, op1=ALU.mult,
                )
                pb2 = psB.tile([C, BC], F32, tag="pb2")
                nc.tensor.matmul(pb2[:], pplusT[:], wb[:], start=True, stop=True)
                pb2_prev = pb2
                if r % RENF == RENF - 1 and r + RENF < MID:
                    rn = r // RENF
                    # fwd renorm (off-chain, DVE)
                    pss = psB.tile([1, BC], F32, tag="pss", bufs=2)
                    nc.tensor.matmul(
                        pss[:], ones20c[:], avec[:, (r + 1) % 2, :],
                        start=True, stop=True,
                    )
                    nc.scalar.activation(sall[0:1, :, rn], pss[:], AF.Copy)
                    srec = wp.tile([1, BC], F32, tag="srec")
                    nc.vector.reciprocal(srec[:], pss[:])
                    pb = psB.tile([C, BC], F32, tag="pb", bufs=1)
                    nc.tensor.matmul(pb[:], ones1[:], srec[:], start=True, stop=True)
                    nc.vector.scalar_tensor_tensor(
                        out=eT[:, ta + RENF, :], in0=eT[:, ta + RENF, :],
                        scalar=0.0, in1=pb[:], op0=ALU.add, op1=ALU.mult,
                    )
                    # bwd renorm (off-chain, Pool), logged from the SBUF wb
                    pss2 = psB.tile([1, BC], F32, tag="pss", bufs=2)
                    nc.tensor.matmul(pss2[:], ones20c[:], wb[:], start=True, stop=True)
                    nc.scalar.activation(sall[0:1, :, 32 + rn], pss2[:], AF.Copy)
                    srec2 = wp.tile([1, BC], F32, tag="srec2")
                    nc.vector.reciprocal(srec2[:], pss2[:])
                    pbb = psB.tile([C, BC], F32, tag="pb", bufs=1)
                    nc.tensor.matmul(pbb[:], ones1[:], srec2[:], start=True, stop=True)
                    nc.vector.scalar_tensor_tensor(
                        out=eT[:, tb - RENF, :], in0=eT[:, tb - RENF, :],
                        scalar=0.0, in1=pbb[:], op0=ALU.add, op1=ALU.mult,
                    )

            # ---- P6: finalization  Z = beta_mid^T alpha_mid ----
            # normalize by colsum(alpha_mid) (logged at slot 31) so the Ln
            # input stays inside the Scalar-engine range
            pssA = psB.tile([1, BC], F32, tag="pss", bufs=2)
            nc.tensor.matmul(
                pssA[:], ones20c[:], avec[:, MID % 2, :], start=True, stop=True
            )
            nc.scalar.activation(sall[0:1, :, 31], pssA[:], AF.Copy)
            srecA = wp.tile([1, BC], F32, tag="srecA")
            nc.vector.reciprocal(srecA[:], pssA[:])
            pbA = psB.tile([C, BC], F32, tag="pb", bufs=1)
            nc.tensor.matmul(pbA[:], ones1[:], srecA[:], start=True, stop=True)
            prodZ = wp.tile([C, BC], F32, tag="prodZ")
            nc.vector.scalar_tensor_tensor(
                out=prodZ[:], in0=pb2_prev[:], scalar=0.0,
                in1=avec[:, MID % 2, :], op0=ALU.add, op1=ALU.mult,
            )
            nc.vector.scalar_tensor_tensor(
                out=prodZ[:], in0=prodZ[:], scalar=0.0,
                in1=pbA[:], op0=ALU.add, op1=ALU.mult,
            )
            paf = psB.tile([1, BC], F32, tag="paf", bufs=1)
            nc.tensor.matmul(paf[:], ones20c[:], prodZ[:], start=True, stop=True)
            flog = wp.tile([1, BC], F32, tag="flog")
            nc.scalar.activation(flog[:], paf[:], AF.Ln)
            slog = wp.tile([1, BC, NSL], F32, tag="slog")
            nc.scalar.activation(slog[:], sall[:], AF.Ln)
            slogsum = wp.tile([1, BC], F32, tag="slogsum")
            nc.vector.tensor_reduce(out=slogsum[:], in_=slog[:], axis=AX.X, op=ALU.add)
            # sum of per-step shifts s_t
            ssum = wp.tile([1, BC], F32, tag="ssum")
            nc.vector.tensor_reduce(
                out=ssum[:],
                in_=s_row[0:1, :].rearrange("a (tl b ti) -> a b tl ti", b=BC, ti=NT),
                axis=AX.XY,
                op=ALU.add,
            )
            # F - gold, sum over batch
            fsum = wp.tile([1, BC], F32, tag="fsum")
            nc.vector.scalar_tensor_tensor(
                out=fsum[:], in0=flog[:], scalar=0.0, in1=slogsum[:], op0=ALU.add, op1=ALU.add,
            )
            nc.vector.scalar_tensor_tensor(
                out=fsum[:], in0=fsum[:], scalar=0.0, in1=ssum[:], op0=ALU.add, op1=ALU.add,
            )
            nc.vector.scalar_tensor_tensor(
                out=fsum[:], in0=fsum[:], scalar=0.0, in1=gold_sb[:], op0=ALU.add, op1=ALU.subtract,
            )
            lp_t = wp.tile([1, 1], F32, tag="lp")
            nc.vector.tensor_reduce(out=lp_t[:], in_=fsum[:], axis=AX.X, op=ALU.add)
            nc.sync.dma_start(out=d_out[:], in_=lp_t[:])
            ps_p5.__exit__(None, None, None)

    nc.finalize()
    return nc


def _prep_inmaps(inputs):
    bf = ml_dtypes.bfloat16
    sent = np.asarray(inputs["sentences"])
    tags = np.asarray(inputs["tags"])
    embed = np.asarray(inputs["embed"], dtype=np.float32)
    trans = np.asarray(inputs["transitions"], dtype=np.float32)
    h0 = np.asarray(inputs["h0"], dtype=np.float32)
    c0 = np.asarray(inputs["c0"], dtype=np.float32)
    W_out = np.asarray(inputs["W_out"], dtype=np.float32)
    b_out = np.asarray(inputs["b_out"], dtype=np.float32)

    rs = np.full((G, 1), 0.5, np.float32)
    rs[2 * H : 3 * H] = 1.0  # g-gate rows unscaled

    embed_bf = np.ascontiguousarray(embed.astype(bf))

    GP = [3, 0, 1, 2]  # gate order (o, i, f, g)

    def chunk_weights(W):  # W [G, K_in] -> [128, 2, CH, 128] = [p, k, c, m]
        Kin = W.shape[1]
        Wr = W.reshape(4, 2, 128, Kin // 128, 128)[GP]  # [gate, hh, m, k, p]
        return np.ascontiguousarray(Wr.transpose(4, 3, 0, 1, 2).reshape(128, Kin // 128, CH, 128))

    wih = np.zeros((128, 2, 2, CH, 128), np.float32)
    whh = np.zeros((128, 2, 2, CH, 128), np.float32)
    xbias = np.zeros((128, 2, CH), np.float32)  # broadcast to BC below
    for d, (Wih, Whh, b) in enumerate(
        [
            (inputs["Wih_f"], inputs["Whh_f"], inputs["b_f"]),
            (inputs["Wih_b"], inputs["Whh_b"], inputs["b_b"]),
        ]
    ):
        Wih = np.asarray(Wih, np.float32) * rs
        Whh = np.asarray(Whh, np.float32) * rs * 0.5
        bt = np.asarray(b, np.float32) * rs[:, 0]
        wih[:, d] = chunk_weights(Wih)
        whh[:, d] = chunk_weights(Whh)
        xbias[:, d] = bt.reshape(4, 2, 128)[GP].transpose(2, 0, 1).reshape(128, CH)
    wih = np.ascontiguousarray(wih.astype(bf))
    whh = np.ascontiguousarray(whh.astype(bf))

    # wout^T [p, d, k, m] = 0.5 * W_out[m, d*256 + k*128 + p]
    wout = np.ascontiguousarray(
        (0.5 * W_out).reshape(C, 2, 2, 128).transpose(3, 1, 2, 0).astype(bf)
    )
    bout = np.ascontiguousarray(b_out[:, None])
    transT = np.ascontiguousarray(trans.T)
    tstop = np.ascontiguousarray(trans[STOP, :][:, None])

    in_maps = []
    for q in range(NCORES):
        bs = slice(q * BC, (q + 1) * BC)
        sq = sent[bs]  # [BC, L]
        tq = tags[bs]
        idx_f = np.ascontiguousarray(
            sq.T.reshape(NT, TPT, BC).transpose(1, 2, 0).reshape(128, NT).astype(np.int32)
        )
        sqr = sq[:, ::-1]
        idx_b = np.ascontiguousarray(
            sqr.T.reshape(NT, TPT, BC).transpose(1, 2, 0).reshape(128, NT).astype(np.int32)
        )
        h0q = np.ascontiguousarray(
            (2.0 * h0[:, bs, :]).reshape(2, BC, 2, 128).transpose(3, 0, 2, 1).astype(bf)
        )
        c0q = np.ascontiguousarray(
            (2.0 * c0[:, bs, :]).reshape(2, BC, 2, 128).transpose(3, 0, 2, 1).astype(np.float32)
        )
        te_prev = np.concatenate(
            [np.full((BC, 1), START, tags.dtype), tq[:, :-1]], axis=1
        )  # prev tag at each t
        ar = np.arange(C)
        ohprev = (ar[:, None, None] == te_prev[None, :, :]).astype(np.float32)
        ohcur = (ar[:, None, None] == tq[None, :, :]).astype(np.float32)
        ohlast = (ar[:, None] == tq[None, :, L - 1]).astype(np.float32)
        a0 = (ar[:, None] == START).astype(np.float32) * np.ones((1, BC), np.float32)
        in_maps.append(
            {
                "embed_bf": embed_bf,
                "idx_f": idx_f,
                "idx_b": idx_b,
                "wih": wih,
                "whh": whh,
                "xbias": xbias,
                "h0T": h0q,
                "c0T": c0q,
                "woutT": wout,
                "bout": bout,
                "transT": transT,
                "tstop": tstop,
                "ohprev": np.ascontiguousarray(ohprev),
                "ohcur": np.ascontiguousarray(ohcur),
                "ohlast": np.ascontiguousarray(ohlast),
                "a0": np.ascontiguousarray(a0),
            }
        )
    return in_maps


def get_module():
    if "nc" not in _CACHE:
        _CACHE["nc"] = _build_module()
    return _CACHE["nc"]


def kernel(**inputs):
    nc = get_module()
    in_maps = _prep_inmaps(inputs)
    res = run_bass_kernel_spmd(nc, in_maps, core_ids=list(range(NCORES)))
    total = sum(float(r["out"][0, 0]) for r in res.results)
    return np.float32(total / B)

